# revision 17
# baseline (speedup 1.0000x reference)
"""Depthwise causal Conv1D (B=4, C=4096, L=4096, K=4) on 8 trn2 NeuronCores.

Sharding: channel-parallel (tensor parallel) — core i owns channels
[i*512, (i+1)*512). Depthwise conv has zero cross-channel interaction, so
there is no communication; each core computes its channel slab end to end.

Per-core kernel layout: channels on SBUF partitions (128 at a time), time on
the free dim. The 4-tap causal FIR along the free dim is computed as four
shifted multiply-accumulate passes with per-partition (per-channel) scalar
weights, split across three engines so no single engine is the bottleneck:

  ScalarE : out[3:L+3]  = w0 * x + bias   (activation, per-partition scale+bias)
            out[0:3]    = bias
  GPSIMD  : out[0:L]   += w3 * x          (scalar_tensor_tensor)
  VectorE : out[1:L+1] += w2 * x          (scalar_tensor_tensor)
            out[2:L+2] += w1 * x          (scalar_tensor_tensor)

DMA (HWDGE via nc.sync) streams 128x4096 fp32 tiles in and 128x4099 tiles
out; the kernel is HBM-bandwidth bound (~64 MB per core total traffic).
"""

import numpy as np

import concourse.bass as bass
import concourse.tile as tile
from concourse import bacc, mybir
from concourse.bass_utils import run_bass_kernel_spmd

B, C, L, K = 4, 4096, 4096, 4
PAD = K - 1
LOUT = L + PAD  # 4099
NCORES = 8
CS = C // NCORES  # 512 channels per core
DT = mybir.dt.float32

_AF = mybir.ActivationFunctionType
_OP = mybir.AluOpType


def build_nc(b=B, cs=CS, l=L, k=K, n_bufs=3, nseg=2):
    """Build the per-core Bass program. Parameterized for small-size sim tests.

    Packs `nseg` batch rows into one [128, nseg*(l+pad)] super-tile with
    `pad` zero columns in front of each segment, so each conv tap is a single
    shifted op spanning all segments (the zero stuffing makes the cross-
    segment spill contribute exactly zero, and the bias-only head columns
    fall out of the main pass automatically).
    """
    ng = cs // 128
    pad = k - 1
    lout = l + pad
    seg = lout  # = pad + l: [pad zeros][l samples]
    w_tile = nseg * seg
    assert b % nseg == 0

    nc = bacc.Bacc("TRN2", target_bir_lowering=False, debug=False, num_devices=NCORES)
    x_d = nc.dram_tensor("x", [b, cs, l], DT, kind="ExternalInput").ap()
    # packed per-channel constants: wb[c] = [w_0..w_{k-1}, bias]
    wb_d = nc.dram_tensor("wb", [cs, k + 1], DT, kind="ExternalInput").ap()
    o_d = nc.dram_tensor("out", [b, cs, lout], DT, kind="ExternalOutput").ap()

    with tile.TileContext(nc) as tc:
        with (
            tc.tile_pool(name="consts", bufs=1) as cpool,
            tc.tile_pool(name="xs", bufs=n_bufs) as xpool,
            tc.tile_pool(name="os", bufs=n_bufs) as opool,
        ):
            # Per-group constant columns: [128, k+1] = w_0..w_{k-1}, bias.
            consts = []
            for g in range(ng):
                ct = cpool.tile([128, k + 1], DT, tag=f"c{g}")
                nc.sync.dma_start(ct[:], wb_d[g * 128 : (g + 1) * 128, :])
                consts.append(ct)

            pending_store = None  # deferred one iteration to keep ACT's HWDGE queue unblocked
            for bi in range(0, b, nseg):
                for g in range(ng):
                    ct = consts[g]
                    c0 = g * 128
                    xt = xpool.tile([128, w_tile], DT, tag="x")
                    xt_seg = xt[:].rearrange("p (s j) -> p s j", s=nseg)
                    # zero-stuff the pad columns at the head of each segment
                    nc.vector.memset(xt_seg[:, :, 0:pad], 0.0)
                    # x[bi:bi+nseg, chans, :] -> per-partition segments
                    nc.sync.dma_start(
                        xt_seg[:, :, pad:seg],
                        x_d[bi : bi + nseg, c0 : c0 + 128, :].rearrange(
                            "s p j -> p s j"
                        ),
                    )
                    ot = opool.tile([128, w_tile], DT, tag="o")

                    # tap 0 (+bias) over the whole super-tile  (ScalarE)
                    nc.scalar.activation(
                        ot[:], xt[:], _AF.Identity,
                        bias=ct[:, k : k + 1], scale=ct[:, 0:1],
                    )
                    if pending_store is not None:
                        dst, src = pending_store
                        nc.scalar.dma_start(dst, src)
                    # taps k-1..1 on VectorE: out[j] += w_t * X[j+t]
                    for t in range(k - 1, 0, -1):
                        nc.vector.scalar_tensor_tensor(
                            out=ot[:, 0 : w_tile - t], in0=xt[:, t:w_tile],
                            scalar=ct[:, t : t + 1], in1=ot[:, 0 : w_tile - t],
                            op0=_OP.mult, op1=_OP.add,
                        )
                    pending_store = (
                        o_d[bi : bi + nseg, c0 : c0 + 128, :].rearrange("s p j -> p s j"),
                        ot[:].rearrange("p (s j) -> p s j", s=nseg),
                    )
            dst, src = pending_store
            nc.scalar.dma_start(dst, src)
    nc.compile()
    return nc


_cached_nc = None


def _get_nc():
    global _cached_nc
    if _cached_nc is None:
        _cached_nc = build_nc()
    return _cached_nc


def run(x, kernel, bias, trace=False, **kwargs):
    """Shard, run on 8 cores, gather. Returns (out, BassKernelResults)."""
    x = np.ascontiguousarray(x, dtype=np.float32)
    w = np.asarray(kernel, dtype=np.float32).reshape(K, C)
    bvec = np.asarray(bias, dtype=np.float32).reshape(C)
    # wb[c] = [w_0[c] .. w_{K-1}[c], bias[c]]
    wb = np.concatenate([w.T, bvec[:, None]], axis=1).astype(np.float32)

    in_maps = []
    for i in range(NCORES):
        sl = slice(i * CS, (i + 1) * CS)
        in_maps.append(
            {
                "x": np.ascontiguousarray(x[:, sl, :]),
                "wb": np.ascontiguousarray(wb[sl, :]),
            }
        )

    nc = _get_nc()
    bkr = run_bass_kernel_spmd(
        nc, in_maps, core_ids=list(range(NCORES)), trace=trace, **kwargs
    )
    out = np.concatenate([r["out"] for r in bkr.results], axis=1)
    return out, bkr


def kernel(x, kernel, bias):
    out, _ = run(x, kernel, bias)
    return out


# revision 19
# speedup vs baseline: 1.1434x; 1.1434x over previous
"""Depthwise causal Conv1D (B=4, C=4096, L=4096, K=4) on 8 trn2 NeuronCores.

Sharding: channel-parallel (tensor parallel) — core i owns channels
[i*512, (i+1)*512). Depthwise conv has zero cross-channel interaction, so
there is no communication; each core computes its channel slab end to end.

Per-core kernel layout: channels on SBUF partitions (128 at a time), time on
the free dim. The 4-tap causal FIR along the free dim is computed as four
shifted multiply-accumulate passes with per-partition (per-channel) scalar
weights, split across three engines so no single engine is the bottleneck:

  ScalarE : out[3:L+3]  = w0 * x + bias   (activation, per-partition scale+bias)
            out[0:3]    = bias
  GPSIMD  : out[0:L]   += w3 * x          (scalar_tensor_tensor)
  VectorE : out[1:L+1] += w2 * x          (scalar_tensor_tensor)
            out[2:L+2] += w1 * x          (scalar_tensor_tensor)

DMA (HWDGE via nc.sync) streams 128x4096 fp32 tiles in and 128x4099 tiles
out; the kernel is HBM-bandwidth bound (~64 MB per core total traffic).
"""

import numpy as np

import concourse.bass as bass
import concourse.tile as tile
from concourse import bacc, mybir
from concourse.bass_utils import run_bass_kernel_spmd

B, C, L, K = 4, 4096, 4096, 4
PAD = K - 1
LOUT = L + PAD  # 4099
NCORES = 8
CS = C // NCORES  # 512 channels per core
DT = mybir.dt.float32

_AF = mybir.ActivationFunctionType
_OP = mybir.AluOpType


def build_nc(b=B, cs=CS, l=L, k=K, n_bufs=6, n_edge_chunks=4):
    """Build the per-core Bass program. Parameterized for small-size sim tests.

    Channels on partitions, time on the free dim. Per [128, l] tile:
    ScalarE writes out[pad:] = w0*x + bias (and bias-only head cols), then
    VectorE folds the remaining taps in with fused scalar_tensor_tensor
    ops (out[pad-t:pad-t+l] += w_t * x). Stores issue from ScalarE's HWDGE,
    deferred one iteration; loads from SP. GpSimd is untouched (its tensor
    ops serialize against VectorE's 2-read ops on the shared SBUF port).

    The first and last tiles are split column-wise into `n_edge_chunks`
    pieces to shorten the pipeline ramp (first ScalarE op starts after a
    fraction of the first load) and drain (last store is a fraction of a
    tile). Each tap maps an x column-chunk to a shifted out range, so the
    chunk decomposition is exact with no halo.
    """
    ng = cs // 128
    pad = k - 1
    lout = l + pad

    nc = bacc.Bacc("TRN2", target_bir_lowering=False, debug=False, num_devices=NCORES)
    x_d = nc.dram_tensor("x", [b, cs, l], DT, kind="ExternalInput").ap()
    # packed per-channel constants: wb[c] = [w_0..w_{k-1}, bias]
    wb_d = nc.dram_tensor("wb", [cs, k + 1], DT, kind="ExternalInput").ap()
    o_d = nc.dram_tensor("out", [b, cs, lout], DT, kind="ExternalOutput").ap()

    with tile.TileContext(nc) as tc:
        with (
            tc.tile_pool(name="consts", bufs=1) as cpool,
            tc.tile_pool(name="xs", bufs=n_bufs) as xpool,
            tc.tile_pool(name="os", bufs=n_bufs) as opool,
        ):
            # Per-group constant columns: [128, k+1] = w_0..w_{k-1}, bias.
            consts = []
            for g in range(ng):
                ct = cpool.tile([128, k + 1], DT, tag=f"c{g}")
                nc.sync.dma_start(ct[:], wb_d[g * 128 : (g + 1) * 128, :])
                consts.append(ct)

            n_tiles = b * ng
            pending_stores = []  # deferred to keep ACT's HWDGE queue unblocked

            def flush_stores():
                for dst, src in pending_stores:
                    nc.scalar.dma_start(dst, src)
                pending_stores.clear()

            ti = 0
            for bi in range(b):
                for g in range(ng):
                    ct = consts[g]
                    c0 = g * 128
                    first, last = ti == 0, ti == n_tiles - 1
                    nchunk = n_edge_chunks if (first or last) else 1
                    cw = l // nchunk

                    xt = xpool.tile([128, l], DT, tag="x")
                    if first:
                        for c in range(nchunk):
                            nc.sync.dma_start(
                                xt[:, c * cw : (c + 1) * cw],
                                x_d[bi, c0 : c0 + 128, c * cw : (c + 1) * cw],
                            )
                    else:
                        nc.sync.dma_start(xt[:], x_d[bi, c0 : c0 + 128, :])
                    ot = opool.tile([128, lout], DT, tag="o")

                    for c in range(nchunk):
                        j0, j1 = c * cw, (c + 1) * cw
                        # tap 0 (+bias): out[pad+j] = w0*x[j] + bias  (ScalarE)
                        nc.scalar.activation(
                            ot[:, pad + j0 : pad + j1], xt[:, j0:j1], _AF.Identity,
                            bias=ct[:, k : k + 1], scale=ct[:, 0:1],
                        )
                        if c == 0:
                            # head columns [0:pad] = bias  (ScalarE)
                            nc.scalar.activation(
                                ot[:, 0:pad], xt[:, 0:pad], _AF.Identity,
                                bias=ct[:, k : k + 1], scale=0.0,
                            )
                            flush_stores()
                        # taps k-1..1 (VectorE): out[pad-t+j] += w_t * x[j]
                        for t in range(k - 1, 0, -1):
                            s = pad - t
                            nc.vector.scalar_tensor_tensor(
                                out=ot[:, s + j0 : s + j1], in0=xt[:, j0:j1],
                                scalar=ct[:, t : t + 1], in1=ot[:, s + j0 : s + j1],
                                op0=_OP.mult, op1=_OP.add,
                            )
                        if last:
                            # store chunk once final: out col m is final after
                            # x cols <= m are folded, i.e. after this chunk for
                            # m in [j0, j1)
                            o1 = lout if c == nchunk - 1 else j1
                            nc.scalar.dma_start(
                                o_d[bi, c0 : c0 + 128, j0:o1], ot[:, j0:o1]
                            )
                    if not last:
                        pending_stores.append(
                            (o_d[bi, c0 : c0 + 128, :], ot[:])
                        )
                    ti += 1
            flush_stores()
    nc.compile()
    return nc


_cached_nc = None


def _get_nc():
    global _cached_nc
    if _cached_nc is None:
        _cached_nc = build_nc()
    return _cached_nc


def run(x, kernel, bias, trace=False, **kwargs):
    """Shard, run on 8 cores, gather. Returns (out, BassKernelResults)."""
    x = np.ascontiguousarray(x, dtype=np.float32)
    w = np.asarray(kernel, dtype=np.float32).reshape(K, C)
    bvec = np.asarray(bias, dtype=np.float32).reshape(C)
    # wb[c] = [w_0[c] .. w_{K-1}[c], bias[c]]
    wb = np.concatenate([w.T, bvec[:, None]], axis=1).astype(np.float32)

    in_maps = []
    for i in range(NCORES):
        sl = slice(i * CS, (i + 1) * CS)
        in_maps.append(
            {
                "x": np.ascontiguousarray(x[:, sl, :]),
                "wb": np.ascontiguousarray(wb[sl, :]),
            }
        )

    nc = _get_nc()
    bkr = run_bass_kernel_spmd(
        nc, in_maps, core_ids=list(range(NCORES)), trace=trace, **kwargs
    )
    out = np.concatenate([r["out"] for r in bkr.results], axis=1)
    return out, bkr


def kernel(x, kernel, bias):
    out, _ = run(x, kernel, bias)
    return out


# revision 24
# speedup vs baseline: 1.4562x; 1.2736x over previous
"""Depthwise causal Conv1D (B=4, C=4096, L=4096, K=4) on 8 trn2 NeuronCores.

Sharding: channel-parallel (tensor parallel) — core i owns channels
[i*512, (i+1)*512). Depthwise conv has zero cross-channel interaction, so
there is no communication; each core computes its channel slab end to end.

Per-core kernel layout: channels on SBUF partitions (128 at a time), time on
the free dim. The 4-tap causal FIR along the free dim is computed as four
shifted multiply-accumulate passes with per-partition (per-channel) scalar
weights, split across three engines so no single engine is the bottleneck:

  ScalarE : out[3:L+3]  = w0 * x + bias   (activation, per-partition scale+bias)
            out[0:3]    = bias
  GPSIMD  : out[0:L]   += w3 * x          (scalar_tensor_tensor)
  VectorE : out[1:L+1] += w2 * x          (scalar_tensor_tensor)
            out[2:L+2] += w1 * x          (scalar_tensor_tensor)

DMA (HWDGE via nc.sync) streams 128x4096 fp32 tiles in and 128x4099 tiles
out; the kernel is HBM-bandwidth bound (~64 MB per core total traffic).
"""

import numpy as np

import concourse.bass as bass
import concourse.tile as tile
from concourse import bacc, mybir
from concourse.bass_utils import run_bass_kernel_spmd

B, C, L, K = 4, 4096, 4096, 4
PAD = K - 1
LOUT = L + PAD  # 4099
NCORES = 8
CS = C // NCORES  # 512 channels per core
DT = mybir.dt.float32

_AF = mybir.ActivationFunctionType
_OP = mybir.AluOpType


def build_nc(b=B, cs=CS, l=L, k=K, n_bufs=5, n_edge_chunks=4, pe_cols=2048):
    """Build the per-core Bass program. Parameterized for small-size sim tests.

    Channels on partitions, time on the free dim. x is loaded into a
    [128, pad + l + pad] tile with `pad` zero columns at both ends
    (xp[i] = x[i - pad]), so every tap reads in-bounds and the causal
    zero-padding falls out of the zero stuffing.

    Work split per tile:
      ScalarE : out[pad:lout] = w0 * xp[pad:lout] + bias; head cols = bias
      PE      : taps 1..k-1 for out cols [0, pe_cols) via diagonal weight
                matmuls accumulating in PSUM (out[m] += sum_t w_t*xp[m+t]),
                512-col chunks, fp32
      VectorE : PSUM chunks merged into out (tensor_tensor add), and
                taps 1..k-1 for out cols [pe_cols, lout) via fused
                scalar_tensor_tensor ops
    Stores issue from ScalarE's HWDGE, deferred one iteration; loads from
    SP. GpSimd stays idle (its tensor ops serialize against VectorE on the
    shared SBUF port pair).

    The first and last tiles are split column-wise into `n_edge_chunks`
    pieces (DVE-only taps) to shorten the pipeline ramp and drain.
    """
    ng = cs // 128
    pad = k - 1
    lout = l + pad
    wx = l + 2 * pad  # padded x width
    assert pe_cols % 512 == 0 and pe_cols + pad <= l

    nc = bacc.Bacc("TRN2", target_bir_lowering=False, debug=False, num_devices=NCORES)
    x_d = nc.dram_tensor("x", [b, cs, l], DT, kind="ExternalInput").ap()
    # packed per-channel constants: wb[c] = [w_0..w_{k-1}, bias]
    wb_d = nc.dram_tensor("wb", [cs, k + 1], DT, kind="ExternalInput").ap()
    eye_d = nc.dram_tensor("eye", [128, 128], DT, kind="ExternalInput").ap()
    o_d = nc.dram_tensor("out", [b, cs, lout], DT, kind="ExternalOutput").ap()

    with tile.TileContext(nc) as tc:
        with (
            tc.tile_pool(name="consts", bufs=1) as cpool,
            tc.tile_pool(name="xs", bufs=n_bufs) as xpool,
            tc.tile_pool(name="os", bufs=n_bufs) as opool,
            tc.tile_pool(name="ps", bufs=6, space="PSUM") as ppool,
        ):
            # Per-group constant columns: [128, k+1] = w_0..w_{k-1}, bias.
            consts = []
            for g in range(ng):
                ct = cpool.tile([128, k + 1], DT, tag=f"c{g}")
                nc.sync.dma_start(ct[:], wb_d[g * 128 : (g + 1) * 128, :])
                consts.append(ct)
            # identity and per-(group, tap) diagonal weight matrices for PE
            diags = {}
            if pe_cols > 0:
                ident = cpool.tile([128, 128], DT, tag="eye")
                nc.sync.dma_start(ident[:], eye_d[:])
                for g in range(ng):
                    for t in range(1, k):
                        dg = cpool.tile([128, 128], DT, tag=f"d{g}_{t}")
                        nc.vector.tensor_scalar(
                            out=dg[:], in0=ident[:], scalar1=consts[g][:, t : t + 1],
                            scalar2=None, op0=_OP.mult,
                        )
                        diags[(g, t)] = dg

            n_tiles = b * ng
            pending_stores = []  # deferred to keep ACT's HWDGE queue unblocked

            def flush_stores():
                for dst, src in pending_stores:
                    nc.scalar.dma_start(dst, src)
                pending_stores.clear()

            ti = 0
            for bi in range(b):
                for g in range(ng):
                    ct = consts[g]
                    c0 = g * 128
                    first, last = ti == 0, ti == n_tiles - 1
                    edge = first or last
                    nchunk = n_edge_chunks if edge else 1
                    cw = l // nchunk
                    n_pe = 0 if edge else pe_cols  # edge tiles are DVE-only

                    xt = xpool.tile([128, wx], DT, tag="x")
                    # zero stuffing: xp[0:pad] = xp[pad+l:] = 0 (GpSimd: tiny,
                    # keeps the VectorE queue free of slot-recycle waits)
                    nc.gpsimd.memset(xt[:, 0:pad], 0.0)
                    nc.gpsimd.memset(xt[:, pad + l : wx], 0.0)
                    if first:
                        for c in range(nchunk):
                            nc.sync.dma_start(
                                xt[:, pad + c * cw : pad + (c + 1) * cw],
                                x_d[bi, c0 : c0 + 128, c * cw : (c + 1) * cw],
                            )
                    else:
                        nc.sync.dma_start(
                            xt[:, pad : pad + l], x_d[bi, c0 : c0 + 128, :]
                        )
                    ot = opool.tile([128, lout], DT, tag="o")

                    for c in range(nchunk):
                        j0, j1 = c * cw, (c + 1) * cw
                        # tap 0 (+bias): out[pad+j] = w0*x[j] + bias  (ScalarE)
                        nc.scalar.activation(
                            ot[:, pad + j0 : pad + j1],
                            xt[:, pad + j0 : pad + j1], _AF.Identity,
                            bias=ct[:, k : k + 1], scale=ct[:, 0:1],
                        )
                        if c == 0:
                            # head columns [0:pad] = bias  (ScalarE)
                            nc.scalar.activation(
                                ot[:, 0:pad], xt[:, 0:pad], _AF.Identity,
                                bias=ct[:, k : k + 1], scale=0.0,
                            )
                            flush_stores()
                        # PE portion: out[m] += sum_t w_t * xp[m+t], m in [0, n_pe)
                        if c == 0 and n_pe > 0:
                            for m0 in range(0, n_pe, 512):
                                pt = ppool.tile([128, 512], DT, tag="p")
                                for t in range(1, k):
                                    nc.tensor.matmul(
                                        pt[:], lhsT=diags[(g, t)][:],
                                        rhs=xt[:, m0 + t : m0 + t + 512],
                                        start=(t == 1), stop=(t == k - 1),
                                    )
                                nc.vector.tensor_tensor(
                                    out=ot[:, m0 : m0 + 512],
                                    in0=pt[:], in1=ot[:, m0 : m0 + 512], op=_OP.add,
                                )
                        # DVE taps: out[m] += w_t * xp[m+t].
                        # On edge tiles, chunk c handles out [j0-pad, j1-pad)
                        # so its tap reads stay within x chunks <= c (xp idx
                        # m+t <= j1-1), keeping the ramp free of forward deps.
                        if edge:
                            m_lo = 0 if c == 0 else j0 - pad
                            m_hi = lout if c == nchunk - 1 else j1 - pad
                        else:
                            m_lo = max(j0, n_pe)
                            m_hi = lout if c == nchunk - 1 else j1
                        if m_hi > m_lo:
                            for t in range(k - 1, 0, -1):
                                nc.vector.scalar_tensor_tensor(
                                    out=ot[:, m_lo:m_hi],
                                    in0=xt[:, m_lo + t : m_hi + t],
                                    scalar=ct[:, t : t + 1],
                                    in1=ot[:, m_lo:m_hi],
                                    op0=_OP.mult, op1=_OP.add,
                                )
                        if last:
                            # store exactly the finalized range of this chunk
                            nc.scalar.dma_start(
                                o_d[bi, c0 : c0 + 128, m_lo:m_hi], ot[:, m_lo:m_hi]
                            )
                    if not last:
                        pending_stores.append((o_d[bi, c0 : c0 + 128, :], ot[:]))
                    ti += 1
            flush_stores()
    nc.compile()
    return nc


_cached_nc = None


def _get_nc():
    global _cached_nc
    if _cached_nc is None:
        _cached_nc = build_nc()
    return _cached_nc


def run(x, kernel, bias, trace=False, **kwargs):
    """Shard, run on 8 cores, gather. Returns (out, BassKernelResults)."""
    x = np.ascontiguousarray(x, dtype=np.float32)
    w = np.asarray(kernel, dtype=np.float32).reshape(K, C)
    bvec = np.asarray(bias, dtype=np.float32).reshape(C)
    # wb[c] = [w_0[c] .. w_{K-1}[c], bias[c]]
    wb = np.concatenate([w.T, bvec[:, None]], axis=1).astype(np.float32)

    eye = np.eye(128, dtype=np.float32)
    in_maps = []
    for i in range(NCORES):
        sl = slice(i * CS, (i + 1) * CS)
        in_maps.append(
            {
                "x": np.ascontiguousarray(x[:, sl, :]),
                "wb": np.ascontiguousarray(wb[sl, :]),
                "eye": eye,
            }
        )

    nc = _get_nc()
    bkr = run_bass_kernel_spmd(
        nc, in_maps, core_ids=list(range(NCORES)), trace=trace, **kwargs
    )
    out = np.concatenate([r["out"] for r in bkr.results], axis=1)
    return out, bkr


def kernel(x, kernel, bias):
    out, _ = run(x, kernel, bias)
    return out
